# revision 7
# baseline (speedup 1.0000x reference)
"""Cross-attention kernel for TRN2 (8 NeuronCores, data-parallel over batch).

Problem (per batch b):
    I_n = LN(image[b]); T_n = LN(text[b])
    attn = softmax(I_n @ T_n^T / sqrt(D), axis=text)
    out_img[b]  = image[b] + attn @ T_n
    out_txt[b]  = text[b]  + attn^T @ I_n

Sharding: batch B=32 split 4-per-core across 8 cores; each core independent
(no collectives).

Per-core kernel layout choices:
  - LN stats via DVE bn_stats/bn_aggr (D on the free axis), apply via one
    fused tensor_scalar (x*rstd + (-mu*rstd)).
  - rstd = exp(-0.5*ln(var+eps)) on ACT so only the ln/exp table set is used
    (avoids sqrt<->exp activation-table thrash with the softmax Exp).
  - QK^T needs both operands with D on partitions: PE-transpose (fp32,
    identity matmul) the normalized tiles; evacuation PSUM->SBUF on ACT.
  - All matmuls in float32r (full PE rate at free-dim 512, fp32 bits).
  - softmax: DVE reduce_max(negate) -> ACT Exp(scale, bias=-scale*max,
    accum_out=row sum) -> DVE reciprocal + tensor_scalar_mul.
  - attn @ T_n needs attn^T: PE-transpose attn tiles, batched 4-wide per
    PSUM bank before evacuation.
  - Residual adds fused into the PSUM evacuation (DVE tensor_add with the
    raw input tiles still resident in SBUF), then one DMA per 128-row block.
"""

import numpy as np

import concourse.bass as bass
import concourse.mybir as mybir
import concourse.tile as tile
from concourse import bacc
from concourse.bass_utils import run_bass_kernel_spmd
from concourse.masks import make_identity

# Problem shapes (hardcoded per contract).
B = 32
NI = 1024  # image tokens
NT = 512  # text tokens
D = 1024  # feature dim
N_CORES = 8
BPC = B // N_CORES  # batches per core
EPS = 1e-5
SCALE = float(D) ** -0.5
P = 128

F32 = mybir.dt.float32
F32R = mybir.dt.float32r
AF = mybir.ActivationFunctionType
AX = mybir.AxisListType

NI_T = NI // P  # 8
NT_T = NT // P  # 4
D_T = D // P  # 8


def build_bass(apply_gb: bool):
    nc = bacc.Bacc("TRN2", target_bir_lowering=False, debug=False, num_devices=N_CORES)

    img = nc.dram_tensor("img", [BPC, NI, D], F32, kind="ExternalInput")
    txt = nc.dram_tensor("txt", [BPC, NT, D], F32, kind="ExternalInput")
    img_out = nc.dram_tensor("img_out", [BPC, NI, D], F32, kind="ExternalOutput")
    txt_out = nc.dram_tensor("txt_out", [BPC, NT, D], F32, kind="ExternalOutput")
    gamma = beta = None
    if apply_gb:
        gamma = nc.dram_tensor("gamma", [D], F32, kind="ExternalInput")
        beta = nc.dram_tensor("beta", [D], F32, kind="ExternalInput")

    # Token-tiled views: token n = tile*128 + p  ->  [b, p, tile, d]
    img_v = img[:].rearrange("b (i p) d -> b p i d", p=P)
    txt_v = txt[:].rearrange("b (t p) d -> b p t d", p=P)
    img_out_v = img_out[:].rearrange("b (i p) d -> b p i d", p=P)
    txt_out_v = txt_out[:].rearrange("b (t p) d -> b p t d", p=P)

    with tile.TileContext(nc) as tc:
        with (
            tc.tile_pool(name="const", bufs=1) as const_pool,
            tc.tile_pool(name="big", bufs=1) as big,
            tc.tile_pool(name="small", bufs=8) as small,
            tc.tile_pool(name="stage", bufs=3) as stage,
            tc.tile_pool(name="ps2", bufs=2, space="PSUM") as ps2,
            tc.tile_pool(name="ps1", bufs=1, space="PSUM") as ps1,
        ):
            ident = const_pool.tile([P, P], F32, name="ident")
            make_identity(nc, ident)
            eps_t = const_pool.tile([P, 1], F32, name="eps_t")
            nc.vector.memset(eps_t, EPS)
            if apply_gb:
                gb_sb = const_pool.tile([2, D], F32, name="gb_sb")
                nc.sync.dma_start(gb_sb[0:1, :], gamma[:][None, :])
                nc.sync.dma_start(gb_sb[1:2, :], beta[:][None, :])

            def layer_norm(raw, xn, n_tiles, pfx):
                """raw, xn: [P, n_tiles, D] SBUF tiles. Writes xn = LN(raw)."""
                for t in range(n_tiles):
                    stats = small.tile([P, 2, 6], F32, tag=f"stats{pfx}")
                    nc.vector.bn_stats(stats[:, 0], raw[:, t, 0:512])
                    nc.vector.bn_stats(stats[:, 1], raw[:, t, 512:1024])
                    mv = small.tile([P, 2], F32, tag=f"mv{pfx}")
                    nc.vector.bn_aggr(mv, stats)
                    # rstd = exp(-0.5 * ln(var + eps))
                    rstd = small.tile([P, 1], F32, tag=f"rstd{pfx}")
                    nc.scalar.activation(rstd, mv[:, 1:2], AF.Ln, bias=eps_t, scale=1.0)
                    nc.scalar.activation(rstd, rstd, AF.Exp, scale=-0.5)
                    # nmr = -(mu * rstd)
                    nmr = small.tile([P, 1], F32, tag=f"nmr{pfx}")
                    nc.vector.tensor_scalar(
                        nmr, mv[:, 0:1], rstd, -1.0,
                        mybir.AluOpType.mult, mybir.AluOpType.mult,
                    )
                    nc.vector.tensor_scalar(
                        xn[:, t, :], raw[:, t, :], rstd, nmr,
                        mybir.AluOpType.mult, mybir.AluOpType.add,
                    )
                    if apply_gb:
                        nc.vector.tensor_tensor(
                            xn[:, t, :], xn[:, t, :],
                            gb_sb[0:1, :].to_broadcast((P, D)),
                            mybir.AluOpType.mult,
                        )
                        nc.vector.tensor_tensor(
                            xn[:, t, :], xn[:, t, :],
                            gb_sb[1:2, :].to_broadcast((P, D)),
                            mybir.AluOpType.add,
                        )

            for b in range(BPC):
                # ---------------- load + LN ----------------
                raw_t = big.tile([P, NT_T, D], F32, tag="raw_t")
                nc.sync.dma_start(raw_t, txt_v[b])
                raw_i = big.tile([P, NI_T, D], F32, tag="raw_i")
                nc.sync.dma_start(raw_i, img_v[b])

                tn = big.tile([P, NT_T, D], F32R, tag="tn")
                layer_norm(raw_t, tn, NT_T, "t")
                inn = big.tile([P, NI_T, D], F32R, tag="inn")
                layer_norm(raw_i, inn, NI_T, "i")

                # ---------------- transposes: tn^T, inn^T ----------------
                # tnT[p, dc, n] = T_n[n, dc*128+p];  inT[p, dc, n] = I_n[n, dc*128+p]
                tnT = big.tile([P, D_T, NT], F32R, tag="tnT")
                for dc in range(D_T):
                    ps = ps2.tile([P, 512], F32, tag="tr", name=f"ps_trt_{b}_{dc}")
                    for t in range(NT_T):
                        nc.tensor.transpose(
                            ps[:, t * P:(t + 1) * P],
                            tn[:, t, dc * P:(dc + 1) * P].bitcast(F32),
                            ident,
                        )
                    nc.scalar.copy(tnT[:, dc, :], ps)

                inT = big.tile([P, D_T, NI], F32R, tag="inT")
                for dc in range(D_T):
                    for half in range(2):
                        ps = ps2.tile(
                            [P, 512], F32, tag="tr", name=f"ps_tri_{b}_{dc}_{half}"
                        )
                        for j in range(4):
                            i = half * 4 + j
                            nc.tensor.transpose(
                                ps[:, j * P:(j + 1) * P],
                                inn[:, i, dc * P:(dc + 1) * P].bitcast(F32),
                                ident,
                            )
                        nc.scalar.copy(inT[:, dc, half * 512:(half + 1) * 512], ps)

                # ---------------- QK^T + softmax (+ attn^T) ----------------
                attn = big.tile([P, NI_T, NT], F32R, tag="attn")
                attnT = big.tile([P, NT_T, NI], F32R, tag="attnT")
                psat = None
                for i in range(NI_T):
                    psl = ps2.tile([P, NT], F32, tag="logits", name=f"ps_l_{b}_{i}")
                    for dc in range(D_T):
                        nc.tensor.matmul(
                            psl,
                            inT[:, dc, i * P:(i + 1) * P],
                            tnT[:, dc, :],
                            start=(dc == 0),
                            stop=(dc == D_T - 1),
                        )
                    nmax = small.tile([P, 1], F32, tag="nmax")
                    nc.vector.reduce_max(nmax, psl, axis=AX.X, negate=True)
                    nsm = small.tile([P, 1], F32, tag="nsm")
                    nc.scalar.mul(nsm, nmax, SCALE)
                    rowsum = small.tile([P, 1], F32, tag="rowsum")
                    nc.scalar.activation(
                        attn[:, i, :], psl, AF.Exp,
                        bias=nsm, scale=SCALE, accum_out=rowsum,
                    )
                    rs = small.tile([P, 1], F32, tag="rs")
                    nc.vector.reciprocal(rs, rowsum)
                    nc.vector.tensor_scalar_mul(attn[:, i, :], attn[:, i, :], rs)

                    # transpose attn tile into per-text-tile PSUM staging
                    if i % 4 == 0:
                        psat = [
                            ps1.tile([P, 512], F32, tag=f"at{t}",
                                     name=f"ps_at_{b}_{i}_{t}")
                            for t in range(NT_T)
                        ]
                    for t in range(NT_T):
                        nc.tensor.transpose(
                            psat[t][:, (i % 4) * P:(i % 4 + 1) * P],
                            attn[:, i, t * P:(t + 1) * P].bitcast(F32),
                            ident,
                        )
                    if i % 4 == 3:
                        half = i // 4
                        for t in range(NT_T):
                            nc.scalar.copy(
                                attnT[:, t, half * 512:(half + 1) * 512], psat[t]
                            )

                # ---------------- attn @ T_n + residual ----------------
                for i in range(NI_T):
                    ost = stage.tile([P, D], F32, tag="ost")
                    for dh in range(2):
                        pso = ps2.tile([P, 512], F32, tag="tr",
                                       name=f"ps_o2_{b}_{i}_{dh}")
                        for t in range(NT_T):
                            nc.tensor.matmul(
                                pso,
                                attnT[:, t, i * P:(i + 1) * P],
                                tn[:, t, dh * 512:(dh + 1) * 512],
                                start=(t == 0),
                                stop=(t == NT_T - 1),
                            )
                        nc.vector.tensor_add(
                            ost[:, dh * 512:(dh + 1) * 512],
                            pso,
                            raw_i[:, i, dh * 512:(dh + 1) * 512],
                        )
                    nc.sync.dma_start(img_out_v[b, :, i, :], ost)

                # ---------------- attn^T @ I_n + residual ----------------
                for t in range(NT_T):
                    ost = stage.tile([P, D], F32, tag="ost")
                    for dh in range(2):
                        pso = ps2.tile([P, 512], F32, tag="logits",
                                       name=f"ps_o3_{b}_{t}_{dh}")
                        for i in range(NI_T):
                            nc.tensor.matmul(
                                pso,
                                attn[:, i, t * P:(t + 1) * P],
                                inn[:, i, dh * 512:(dh + 1) * 512],
                                start=(i == 0),
                                stop=(i == NI_T - 1),
                            )
                        nc.vector.tensor_add(
                            ost[:, dh * 512:(dh + 1) * 512],
                            pso,
                            raw_t[:, t, dh * 512:(dh + 1) * 512],
                        )
                    nc.sync.dma_start(txt_out_v[b, :, t, :], ost)

    nc.compile()
    return nc


_NC_CACHE = {}


def _get_nc(apply_gb: bool):
    if apply_gb not in _NC_CACHE:
        _NC_CACHE[apply_gb] = build_bass(apply_gb)
    return _NC_CACHE[apply_gb]


def run(image_features, text_features, gamma, beta, trace=False):
    image_features = np.ascontiguousarray(np.asarray(image_features, dtype=np.float32))
    text_features = np.ascontiguousarray(np.asarray(text_features, dtype=np.float32))
    gamma = np.asarray(gamma, dtype=np.float32)
    beta = np.asarray(beta, dtype=np.float32)
    apply_gb = not (np.all(gamma == 1.0) and np.all(beta == 0.0))

    nc = _get_nc(apply_gb)

    in_maps = []
    for c in range(N_CORES):
        m = {
            "img": image_features[c * BPC:(c + 1) * BPC],
            "txt": text_features[c * BPC:(c + 1) * BPC],
        }
        if apply_gb:
            m["gamma"] = gamma
            m["beta"] = beta
        in_maps.append(m)

    res = run_bass_kernel_spmd(nc, in_maps, core_ids=list(range(N_CORES)), trace=trace)
    img_out = np.concatenate([r["img_out"] for r in res.results], axis=0)
    txt_out = np.concatenate([r["txt_out"] for r in res.results], axis=0)
    return (img_out, txt_out), res


def kernel(image_features, text_features, gamma, beta):
    (img_out, txt_out), _ = run(image_features, text_features, gamma, beta)
    return img_out, txt_out
